# revision 8
# baseline (speedup 1.0000x reference)
"""Trainium2 Bass kernel for nn_CommunicationNetwork (masked deep-sets MLP).

Pipeline per batch element:
  shared MLP 17->256->256->128 (relu) on 37 sequence rows, masked sum-pool,
  concat with 9 own-features, actor head 137->256->256->23 and value head
  137->256->256->1.

Strategy:
  - Pure data parallel: batch 16384 -> 2048 per NeuronCore (8 cores).
  - Host packs inputs feature-major: xT [18, 37*2048] (feature 17 = const 1.0
    so layer-1 bias rides inside the matmul), rows ordered r = s*2048 + b.
  - All matmuls run with float32r operands (fp22, full PE rate at N=512);
    accumulation fp32 in PSUM.
  - Padded (all-zero) rows are NOT masked on device: they produce the
    weight-only constant c = MLP(0).  The masked sum equals the plain sum
    minus npad*c, applied as one rank-1 matmul per 512-batch block (c is
    computed on device from the weights; npad counted on host).
  - Evictions (PSUM->SBUF, relu+bias) are split across ScalarE and VectorE;
    sum-pooling over the 37 rows runs on GPSIMD so it overlaps with PE.
  - All constants ride in two packed blob DMAs (weights f32r / biases f32)
    to keep per-instruction semaphore fan-in low.
"""

import numpy as np

_B, _S, _F, _OWN, _H, _E = 16384, 37, 17, 9, 256, 128
_NOUT = 23
_NCORES = 8
_Bc = _B // _NCORES          # 2048 batch per core
_R = _S * _Bc                # 75776 sequence rows per core
_XCOLS = 8192                # xT columns per DMA tile (4 s-values)
_CH = 512                    # matmul free-dim chunk

# (name, partitions, columns) slots inside the two constant blobs
_WSLOTS = [
    ("w1", 18, 256),
    ("w2a", 128, 256), ("w2b", 128, 256),
    ("w3a", 128, 128), ("w3b", 128, 128),
    ("aw1x", 128, 256), ("aw1o", 10, 256),
    ("vw1x", 128, 256), ("vw1o", 10, 256),
    ("aw2a", 128, 256), ("aw2b", 128, 256),
    ("vw2a", 128, 256), ("vw2b", 128, 256),
    ("aw3a", 128, _NOUT), ("aw3b", 128, _NOUT),
    ("vw3a", 128, 1), ("vw3b", 128, 1),
    ("nneg", 1, _Bc), ("own", 10, _Bc),
]
_BSLOTS = [
    ("sb1a", 128, 1), ("sb1b", 128, 1),
    ("sb2a", 128, 1), ("sb2b", 128, 1),
    ("sb3c", 128, 1),
    ("ab2a", 128, 1), ("ab2b", 128, 1),
    ("vb2a", 128, 1), ("vb2b", 128, 1),
    ("ab3c", _NOUT, 1), ("vb3c", 1, 1),
    ("sb3r", 1, 128),
    ("sb1a2", 128, 2), ("sb1b2", 128, 2),
]


def _slot_offsets(slots):
    offs, c = {}, 0
    for name, p, w in slots:
        offs[name] = (c, p, w)
        c += w
    return offs, c


_WOFF, _WTOT = _slot_offsets(_WSLOTS)
_BOFF, _BTOT = _slot_offsets(_BSLOTS)

LAST_RESULTS = None
_NC_CACHE = None
_EXEC = None


def _build():
    import concourse.mybir as mybir
    from concourse import bacc
    from concourse.tile import TileContext

    dt = mybir.dt
    f32 = dt.float32
    f32r = dt.float32r
    AF = mybir.ActivationFunctionType
    OP = mybir.AluOpType

    nc = bacc.Bacc(trn_type="TRN2", debug=False, enable_partition_id=False)

    x = nc.dram_tensor("x", [18, _R], f32r, kind="ExternalInput")
    wblob = nc.dram_tensor("wblob", [128, _WTOT], f32r, kind="ExternalInput")
    bblob = nc.dram_tensor("bblob", [128, _BTOT], f32, kind="ExternalInput")
    out = nc.dram_tensor("out", [_NOUT, _Bc], f32, kind="ExternalOutput")
    outv = nc.dram_tensor("outv", [1, _Bc], f32, kind="ExternalOutput")

    with TileContext(nc) as tc:
        with (
            tc.tile_pool(name="const", bufs=1) as cp,
            tc.tile_pool(name="xin", bufs=2) as xp,
            tc.tile_pool(name="act", bufs=3) as actp,
            tc.tile_pool(name="ps1", bufs=1, space="PSUM") as ps1,
            tc.tile_pool(name="ps2", bufs=2, space="PSUM") as ps2,
            tc.tile_pool(name="ps3", bufs=2, space="PSUM") as ps3,
        ):
            wt = cp.tile([128, _WTOT], f32r, tag="wblob")
            bt = cp.tile([128, _BTOT], f32, tag="bblob")
            nc.sync.dma_start(wt[:], wblob[:])
            nc.sync.dma_start(bt[:], bblob[:])

            def W(name):
                c, p, w = _WOFF[name]
                return wt[0:p, c:c + w]

            def Bv(name):
                c, p, w = _BOFF[name]
                return bt[0:p, c:c + w]

            tw1, tw2a, tw2b, tw3a, tw3b = (W(n) for n in
                                           ("w1", "w2a", "w2b", "w3a", "w3b"))
            tsb2a, tsb2b, tsb3c = Bv("sb2a"), Bv("sb2b"), Bv("sb3c")
            town, tnneg = W("own"), W("nneg")

            pooled = cp.tile([128, _Bc], f32r, tag="pooled")
            outT = cp.tile([_NOUT, _Bc], f32, tag="outT")
            outV = cp.tile([1, _Bc], f32, tag="outV")

            # ---- phase A: cT[1,128] = relu-MLP(zero row), from weights ----
            # relu(0*in + bias); width-2 tiles because f32r matmul needs N>=2
            h1c0 = cp.tile([128, 2], f32r, tag="h1c0")
            h1c1 = cp.tile([128, 2], f32r, tag="h1c1")
            nc.scalar.activation(h1c0[:], Bv("sb1a2"), AF.Relu,
                                 bias=Bv("sb1a"), scale=0.0)
            nc.scalar.activation(h1c1[:], Bv("sb1b2"), AF.Relu,
                                 bias=Bv("sb1b"), scale=0.0)
            h2c0 = cp.tile([128, 2], f32r, tag="h2c0")
            h2c1 = cp.tile([128, 2], f32r, tag="h2c1")
            for m, h2cm, sb2m in ((0, h2c0, tsb2a), (1, h2c1, tsb2b)):
                z2c = ps3.tile([128, 2], f32, tag="l3")
                ms = slice(128 * m, 128 * (m + 1))
                nc.tensor.matmul(z2c[:], tw2a[:, ms], h1c0[:],
                                 start=True, stop=False)
                nc.tensor.matmul(z2c[:], tw2b[:, ms], h1c1[:],
                                 start=False, stop=True)
                nc.scalar.activation(h2cm[:], z2c[:], AF.Relu, bias=sb2m[:])
            zc = ps3.tile([2, 128], f32, tag="l3")
            nc.tensor.matmul(zc[:], h2c0[:], tw3a[:], start=True, stop=False)
            nc.tensor.matmul(zc[:], h2c1[:], tw3b[:], start=False, stop=True)
            ctt = cp.tile([1, 128], f32, tag="ctt")
            ct = cp.tile([1, 128], f32r, tag="ct")
            nc.vector.tensor_tensor(ctt[:], zc[0:1, :], Bv("sb3r"), op=OP.add)
            nc.vector.tensor_scalar_max(ct[:], ctt[:], 0.0)

            # ---- main loop: shared MLP over all rows + pooling ----
            n_tiles = (_R + _XCOLS - 1) // _XCOLS
            chunk_idx = 0
            for t in range(n_tiles):
                c0g = t * _XCOLS
                ncols = min(_XCOLS, _R - c0g)
                xt = xp.tile([18, _XCOLS], f32r, tag="xt")
                nc.sync.dma_start(xt[:, :ncols], x[:, c0g:c0g + ncols])
                for j in range(ncols // _CH):
                    lc = j * _CH
                    gcol = c0g + lc
                    s = gcol // _Bc
                    blk = (gcol % _Bc) // _CH
                    xs = xt[:, lc:lc + _CH]

                    pl1 = ps1.tile([128, 1024], f32, tag="l1")
                    nc.tensor.matmul(pl1[:, 0:512], tw1[:, 0:128], xs,
                                     start=True, stop=True)
                    nc.tensor.matmul(pl1[:, 512:1024], tw1[:, 128:256], xs,
                                     start=True, stop=True)
                    h1 = actp.tile([128, 1024], f32r, tag="h1")
                    nc.scalar.activation(h1[:], pl1[:], AF.Relu)

                    pl2 = ps2.tile([128, 1024], f32, tag="l2")
                    for m in (0, 1):
                        ms = slice(128 * m, 128 * (m + 1))
                        os_ = slice(512 * m, 512 * (m + 1))
                        nc.tensor.matmul(pl2[:, os_], tw2a[:, ms],
                                         h1[:, 0:512], start=True, stop=False)
                        nc.tensor.matmul(pl2[:, os_], tw2b[:, ms],
                                         h1[:, 512:1024], start=False, stop=True)
                    h2 = actp.tile([128, 1024], f32r, tag="h2")
                    nc.vector.tensor_scalar(h2[:, 0:512], pl2[:, 0:512],
                                            tsb2a[:], 0.0, OP.add, OP.max)
                    nc.vector.tensor_scalar(h2[:, 512:1024], pl2[:, 512:1024],
                                            tsb2b[:], 0.0, OP.add, OP.max)

                    pl3 = ps3.tile([128, 512], f32, tag="l3")
                    nc.tensor.matmul(pl3[:], tw3a[:], h2[:, 0:512],
                                     start=True, stop=False)
                    nc.tensor.matmul(pl3[:], tw3b[:], h2[:, 512:1024],
                                     start=False, stop=True)

                    dst = pooled[:, blk * _CH:(blk + 1) * _CH]
                    if s == 0:
                        nc.scalar.activation(dst, pl3[:], AF.Relu, bias=tsb3c[:])
                    else:
                        h3 = actp.tile([128, 512], f32r, tag="h3")
                        if chunk_idx % 5 == 4:
                            nc.vector.tensor_scalar(h3[:], pl3[:], tsb3c[:],
                                                    0.0, OP.add, OP.max)
                        else:
                            nc.scalar.activation(h3[:], pl3[:], AF.Relu,
                                                 bias=tsb3c[:])
                        nc.gpsimd.tensor_tensor(dst, dst, h3[:], op=OP.add)
                    chunk_idx += 1

            # ---- correction + heads per 512-batch block ----
            for blk in range(_Bc // _CH):
                bs = slice(blk * _CH, (blk + 1) * _CH)
                psc = ps3.tile([128, 512], f32, tag="l3")
                nc.tensor.matmul(psc[:], ct[:], tnneg[:, bs],
                                 start=True, stop=True)
                nc.vector.tensor_tensor(pooled[:, bs], pooled[:, bs], psc[:],
                                        op=OP.add)

                for (w1x, w1o, hw2a, hw2b, hb2a, hb2b, hw3a, hw3b, hb3,
                     onum) in (
                        (W("aw1x"), W("aw1o"), W("aw2a"), W("aw2b"),
                         Bv("ab2a"), Bv("ab2b"), W("aw3a"), W("aw3b"),
                         Bv("ab3c"), _NOUT),
                        (W("vw1x"), W("vw1o"), W("vw2a"), W("vw2b"),
                         Bv("vb2a"), Bv("vb2b"), W("vw3a"), W("vw3b"),
                         Bv("vb3c"), 1)):
                    pA = ps1.tile([128, 1024], f32, tag="l1")
                    for m in (0, 1):
                        ms = slice(128 * m, 128 * (m + 1))
                        os_ = slice(512 * m, 512 * (m + 1))
                        nc.tensor.matmul(pA[:, os_], w1x[:, ms],
                                         pooled[:, bs], start=True, stop=False)
                        nc.tensor.matmul(pA[:, os_], w1o[:, ms],
                                         town[:, bs], start=False, stop=True)
                    s1 = actp.tile([128, 1024], f32r, tag="h1")
                    nc.scalar.activation(s1[:], pA[:], AF.Relu)

                    pB = ps2.tile([128, 1024], f32, tag="l2")
                    for m in (0, 1):
                        ms = slice(128 * m, 128 * (m + 1))
                        os_ = slice(512 * m, 512 * (m + 1))
                        nc.tensor.matmul(pB[:, os_], hw2a[:, ms],
                                         s1[:, 0:512], start=True, stop=False)
                        nc.tensor.matmul(pB[:, os_], hw2b[:, ms],
                                         s1[:, 512:1024], start=False, stop=True)
                    s2 = actp.tile([128, 1024], f32r, tag="h2")
                    nc.vector.tensor_scalar(s2[:, 0:512], pB[:, 0:512],
                                            hb2a[:], 0.0, OP.add, OP.max)
                    nc.vector.tensor_scalar(s2[:, 512:1024], pB[:, 512:1024],
                                            hb2b[:], 0.0, OP.add, OP.max)

                    pC = ps3.tile([onum, 512], f32, tag="l3")
                    nc.tensor.matmul(pC[:], hw3a[:, 0:onum], s2[:, 0:512],
                                     start=True, stop=False)
                    nc.tensor.matmul(pC[:], hw3b[:, 0:onum], s2[:, 512:1024],
                                     start=False, stop=True)
                    odst = outT if onum == _NOUT else outV
                    nc.scalar.activation(odst[0:onum, bs], pC[:],
                                         AF.Identity, bias=hb3[:])

            nc.sync.dma_start(out[:, :], outT[:])
            nc.sync.dma_start(outv[:, :], outV[:])

    nc.finalize()
    return nc


def _host_prep(inputs):
    ins = {k: np.ascontiguousarray(np.asarray(v), dtype=np.float32)
           for k, v in inputs.items()}
    obs = ins["obs"]
    seqs = obs[:, 1:, :]
    own = obs[:, 0, :_OWN]
    valid = (np.abs(seqs).sum(-1) != 0)
    negnpad = -(_S - valid.sum(1)).astype(np.float32)

    wvals = {
        "w1": np.concatenate([ins["sW1"], ins["sb1"][None, :]], 0),
        "w2a": ins["sW2"][:128], "w2b": ins["sW2"][128:],
        "w3a": ins["sW3"][:128], "w3b": ins["sW3"][128:],
        "aw1x": ins["aW1"][:128],
        "aw1o": np.concatenate([ins["aW1"][128:], ins["ab1"][None, :]], 0),
        "vw1x": ins["vW1"][:128],
        "vw1o": np.concatenate([ins["vW1"][128:], ins["vb1"][None, :]], 0),
        "aw2a": ins["aW2"][:128], "aw2b": ins["aW2"][128:],
        "vw2a": ins["vW2"][:128], "vw2b": ins["vW2"][128:],
        "aw3a": ins["aW3"][:128], "aw3b": ins["aW3"][128:],
        "vw3a": ins["vW3"][:128], "vw3b": ins["vW3"][128:],
    }
    bvals = {
        "sb1a": ins["sb1"][:128, None], "sb1b": ins["sb1"][128:, None],
        "sb2a": ins["sb2"][:128, None], "sb2b": ins["sb2"][128:, None],
        "sb3c": ins["sb3"][:, None],
        "ab2a": ins["ab2"][:128, None], "ab2b": ins["ab2"][128:, None],
        "vb2a": ins["vb2"][:128, None], "vb2b": ins["vb2"][128:, None],
        "ab3c": ins["ab3"][:, None], "vb3c": ins["vb3"][:, None],
        "sb3r": ins["sb3"][None, :],
        "sb1a2": np.repeat(ins["sb1"][:128, None], 2, axis=1),
        "sb1b2": np.repeat(ins["sb1"][128:, None], 2, axis=1),
    }

    bblob = np.zeros((128, _BTOT), np.float32)
    for name, (c, p, w) in _BOFF.items():
        bblob[:p, c:c + w] = bvals[name]

    in_maps = []
    for cid in range(_NCORES):
        sl = slice(cid * _Bc, (cid + 1) * _Bc)
        xT = np.empty((18, _R), np.float32)
        xT[:17] = seqs[sl].transpose(2, 1, 0).reshape(17, _R)
        xT[17] = 1.0

        wblob = np.zeros((128, _WTOT), np.float32)
        for name, (c, p, w) in _WOFF.items():
            if name == "nneg":
                wblob[:1, c:c + w] = negnpad[sl][None, :]
            elif name == "own":
                wblob[:9, c:c + w] = own[sl].T
                wblob[9, c:c + w] = 1.0
            else:
                wblob[:p, c:c + w] = wvals[name]
        in_maps.append({"x": xT, "wblob": wblob, "bblob": bblob})
    return in_maps


def _get_exec():
    """Build (once) a jitted shard_map executing the bass NEFF on 8 cores."""
    global _EXEC, _NC_CACHE
    if _EXEC is not None:
        return _EXEC
    import jax
    from jax.sharding import Mesh, NamedSharding, PartitionSpec
    from jax.experimental.shard_map import shard_map
    from concourse import bass2jax, mybir

    bass2jax.install_neuronx_cc_hook()
    if _NC_CACHE is None:
        _NC_CACHE = _build()
    nc = _NC_CACHE

    in_names, out_names, out_avals, zero_outs = [], [], [], []
    for alloc in nc.m.functions[0].allocations:
        if not isinstance(alloc, mybir.MemoryLocationSet):
            continue
        name = alloc.memorylocations[0].name
        if alloc.kind == "ExternalInput":
            in_names.append(name)
        elif alloc.kind == "ExternalOutput":
            out_names.append(name)
            shape = tuple(alloc.tensor_shape)
            dtype = mybir.dt.np(alloc.dtype)
            out_avals.append(jax.core.ShapedArray(shape, dtype))
            zero_outs.append(np.zeros(shape, dtype))
    all_names = tuple(in_names + out_names)

    def _body(*args):
        outs = bass2jax._bass_exec_p.bind(
            *args,
            out_avals=tuple(out_avals),
            in_names=all_names,
            out_names=tuple(out_names),
            lowering_input_output_aliases=(),
            sim_require_finite=True,
            sim_require_nnan=True,
            nc=nc,
        )
        return tuple(outs)

    devices = jax.devices()[:_NCORES]
    mesh = Mesh(np.asarray(devices), ("core",))
    nin = len(in_names) + len(out_names)
    fn = jax.jit(
        shard_map(_body, mesh=mesh,
                  in_specs=(PartitionSpec("core"),) * nin,
                  out_specs=(PartitionSpec("core"),) * len(out_names),
                  check_rep=False),
        keep_unused=True)
    shard = NamedSharding(mesh, PartitionSpec("core"))
    _EXEC = (fn, in_names, out_names, zero_outs, shard)
    return _EXEC


def _device_inputs(in_maps):
    import jax
    fn, in_names, out_names, zero_outs, shard = _get_exec()
    concat = [np.concatenate([np.asarray(m[n]) for m in in_maps], axis=0)
              for n in in_names]
    concat += [np.concatenate([z] * _NCORES, axis=0) for z in zero_outs]
    return [jax.device_put(a, shard) for a in concat]


def _run_device(args_dev):
    fn, in_names, out_names, zero_outs, shard = _get_exec()
    outs = fn(*args_dev)
    return {n: np.asarray(o) for n, o in zip(out_names, outs)}


def kernel(**inputs):
    in_maps = _host_prep(inputs)
    args_dev = _device_inputs(in_maps)
    outs = _run_device(args_dev)
    o = outs["out"].reshape(_NCORES, _NOUT, _Bc)
    ov = outs["outv"].reshape(_NCORES, 1, _Bc)
    logits = np.concatenate([o[c].T for c in range(_NCORES)], 0)
    value = np.concatenate([ov[c, 0] for c in range(_NCORES)], 0)
    return (np.ascontiguousarray(logits, dtype=np.float32),
            np.ascontiguousarray(value, dtype=np.float32))


def benchmark(inputs, reps=20):
    """Warm per-call wall-clock times (seconds) of the device execution."""
    import time
    in_maps = _host_prep(inputs)
    args_dev = _device_inputs(in_maps)
    fn = _get_exec()[0]
    ts = []
    for _ in range(reps + 1):
        t0 = time.perf_counter()
        outs = fn(*args_dev)
        for o in outs:
            o.block_until_ready()
        ts.append(time.perf_counter() - t0)
    return ts[1:]


# revision 12
# speedup vs baseline: 13656.0062x; 13656.0062x over previous
"""Trainium2 Bass kernel for nn_CommunicationNetwork (masked deep-sets MLP).

Pipeline per batch element:
  shared MLP 17->256->256->128 (relu) on 37 sequence rows, masked sum-pool,
  concat with 9 own-features, actor head 137->256->256->23 and value head
  137->256->256->1.

Strategy:
  - Pure data parallel: batch 16384 -> 2048 per NeuronCore (8 cores).
  - Host packs inputs feature-major: xT [18, 37*2048] (feature 17 = const 1.0
    so layer-1 bias rides inside the matmul), rows ordered r = s*2048 + b.
  - All matmuls run with float32r operands (fp22, full PE rate at N=512);
    accumulation fp32 in PSUM.
  - Padded (all-zero) rows are NOT masked on device: they produce the
    weight-only constant c = MLP(0).  The masked sum equals the plain sum
    minus npad*c, applied as one rank-1 matmul per 512-batch block (c is
    computed on device from the weights; npad counted on host).
  - Evictions (PSUM->SBUF, relu+bias) are split across ScalarE and VectorE;
    sum-pooling over the 37 rows runs on GPSIMD so it overlaps with PE.
  - All constants ride in two packed blob DMAs (weights f32r / biases f32)
    to keep per-instruction semaphore fan-in low.
"""

import os

import numpy as np

_B, _S, _F, _OWN, _H, _E = 16384, 37, 17, 9, 256, 128
_NOUT = 23
_NCORES = 8
_Bc = _B // _NCORES          # 2048 batch per core
_R = _S * _Bc                # 75776 sequence rows per core
_XCOLS = 8192                # xT columns per DMA tile (4 s-values)
_CH = 512                    # matmul free-dim chunk

# (name, partitions, columns) slots inside the packed constant blobs
_WSLOTS = [
    ("w2a", 128, 256), ("w2b", 128, 256),
    ("w3a", 128, 128), ("w3b", 128, 128),
    ("aw1x", 128, 256), ("aw1o", 10, 256),
    ("vw1x", 128, 256), ("vw1o", 10, 256),
    ("aw2a", 128, 256), ("aw2b", 128, 256),
    ("vw2a", 128, 256), ("vw2b", 128, 256),
    ("aw3a", 128, _NOUT), ("aw3b", 128, _NOUT),
    ("vw3a", 128, 1), ("vw3b", 128, 1),
]
_BSLOTS = [
    ("sb1a", 128, 1), ("sb1b", 128, 1),
    ("sb2a", 128, 1), ("sb2b", 128, 1),
    ("sb3c", 128, 1),
    ("ab2a", 128, 1), ("ab2b", 128, 1),
    ("vb2a", 128, 1), ("vb2b", 128, 1),
    ("ab3c", _NOUT, 1), ("vb3c", 1, 1),
    ("sb3r", 1, 128),
    ("sb1a2", 128, 2), ("sb1b2", 128, 2),
]


def _slot_offsets(slots):
    offs, c = {}, 0
    for name, p, w in slots:
        offs[name] = (c, p, w)
        c += w
    return offs, c


_WOFF, _WTOT = _slot_offsets(_WSLOTS)
_BOFF, _BTOT = _slot_offsets(_BSLOTS)

LAST_RESULTS = None
_NC_CACHE = None
_EXEC = None


def _build():
    import concourse.mybir as mybir
    from concourse import bacc
    from concourse.tile import TileContext

    dt = mybir.dt
    f32 = dt.float32
    f32r = dt.float32r
    AF = mybir.ActivationFunctionType
    OP = mybir.AluOpType

    nc = bacc.Bacc(trn_type="TRN2", debug=False, enable_partition_id=False)

    x = nc.dram_tensor("x", [18, _R], f32r, kind="ExternalInput")
    w1d = nc.dram_tensor("w1d", [18, 256], f32r, kind="ExternalInput")
    wblob = nc.dram_tensor("wblob", [128, _WTOT], f32r, kind="ExternalInput")
    bblob = nc.dram_tensor("bblob", [128, _BTOT], f32, kind="ExternalInput")
    hblob = nc.dram_tensor("hblob", [10, 2 * _Bc], f32r, kind="ExternalInput")
    out = nc.dram_tensor("out", [_NOUT, _Bc], f32, kind="ExternalOutput")
    outv = nc.dram_tensor("outv", [1, _Bc], f32, kind="ExternalOutput")

    NCH = _R // _CH              # 148 chunks of 512 rows
    GRP = _XCOLS // _CH          # 16 chunks per xt tile

    with TileContext(nc) as tc:
        _e = lambda k, d: int(os.environ.get(k, d))
        evm, evk = _e("K_EVM", 5), _e("K_EVK", 4)
        with (
            tc.tile_pool(name="const", bufs=1) as cp,
            tc.tile_pool(name="xin", bufs=_e("K_XT", 2)) as xp,
            tc.tile_pool(name="act", bufs=_e("K_ACT", 3)) as actp,
            tc.tile_pool(name="ps1", bufs=_e("K_PS1", 1), space="PSUM") as ps1,
            tc.tile_pool(name="ps2", bufs=_e("K_PS2", 2), space="PSUM") as ps2,
            tc.tile_pool(name="ps3", bufs=_e("K_PS3", 2), space="PSUM") as ps3,
        ):
            w1t = cp.tile([18, 256], f32r, tag="w1t")
            bt = cp.tile([128, _BTOT], f32, tag="bblob")
            wt = cp.tile([128, _WTOT], f32r, tag="wblob")
            ht = cp.tile([10, 2 * _Bc], f32r, tag="hblob")
            nc.sync.dma_start(w1t[:], w1d[:])
            nc.sync.dma_start(bt[:], bblob[:])

            def W(name):
                c, p, w = _WOFF[name]
                return wt[0:p, c:c + w]

            def Bv(name):
                c, p, w = _BOFF[name]
                return bt[0:p, c:c + w]

            tw1 = w1t[:, :]
            tw2a, tw2b, tw3a, tw3b = (W(n) for n in
                                      ("w2a", "w2b", "w3a", "w3b"))
            tsb2a, tsb2b, tsb3c = Bv("sb2a"), Bv("sb2b"), Bv("sb3c")
            town = ht[0:10, 0:_Bc]
            tnneg = ht[0:1, _Bc:2 * _Bc]

            pooled = cp.tile([128, _Bc], f32r, tag="pooled")
            outT = cp.tile([_NOUT, _Bc], f32, tag="outT")
            outV = cp.tile([1, _Bc], f32, tag="outV")
            ct = cp.tile([1, 128], f32r, tag="ct")

            def emit_phase_a():
                # cT[1,128] = relu-MLP(zero row); width-2 (f32r needs N>=2)
                h1c0 = cp.tile([128, 2], f32r, tag="h1c0")
                h1c1 = cp.tile([128, 2], f32r, tag="h1c1")
                nc.scalar.activation(h1c0[:], Bv("sb1a2"), AF.Relu,
                                     bias=Bv("sb1a"), scale=0.0)
                nc.scalar.activation(h1c1[:], Bv("sb1b2"), AF.Relu,
                                     bias=Bv("sb1b"), scale=0.0)
                h2c0 = cp.tile([128, 2], f32r, tag="h2c0")
                h2c1 = cp.tile([128, 2], f32r, tag="h2c1")
                for m, h2cm, sb2m in ((0, h2c0, tsb2a), (1, h2c1, tsb2b)):
                    z2c = ps3.tile([128, 2], f32, tag="l3")
                    ms = slice(128 * m, 128 * (m + 1))
                    nc.tensor.matmul(z2c[:], tw2a[:, ms], h1c0[:],
                                     start=True, stop=False)
                    nc.tensor.matmul(z2c[:], tw2b[:, ms], h1c1[:],
                                     start=False, stop=True)
                    nc.scalar.activation(h2cm[:], z2c[:], AF.Relu, bias=sb2m[:])
                zc = ps3.tile([2, 128], f32, tag="l3")
                nc.tensor.matmul(zc[:], h2c0[:], tw3a[:], start=True, stop=False)
                nc.tensor.matmul(zc[:], h2c1[:], tw3b[:], start=False, stop=True)
                ctt = cp.tile([1, 128], f32, tag="ctt")
                nc.vector.tensor_tensor(ctt[:], zc[0:1, :], Bv("sb3r"), op=OP.add)
                nc.vector.tensor_scalar_max(ct[:], ctt[:], 0.0)

            # ---- software-pipelined main loop ----
            # iteration i: L1(i) | L2(i-1) | L3+evict+pool(i-2)
            xts = {}
            st = {}

            def emit_l1(i):
                g = i // GRP
                if g not in xts:
                    c0g = g * _XCOLS
                    ncols = min(_XCOLS, _R - c0g)
                    xt = xp.tile([18, _XCOLS], f32r, tag="xt")
                    if g == 0:
                        # split so the first chunks' data lands early
                        for q in range(4):
                            qs = q * (_XCOLS // 4)
                            nc.sync.dma_start(xt[:, qs:qs + _XCOLS // 4],
                                              x[:, qs:qs + _XCOLS // 4])
                        nc.sync.dma_start(wt[:], wblob[:])
                        nc.sync.dma_start(ht[:], hblob[:])
                    else:
                        nc.sync.dma_start(xt[:, :ncols], x[:, c0g:c0g + ncols])
                    xts[g] = xt
                lc = (i % GRP) * _CH
                xs = xts[g][:, lc:lc + _CH]
                pl1 = ps1.tile([128, 1024], f32, tag="l1")
                nc.tensor.matmul(pl1[:, 0:512], tw1[:, 0:128], xs,
                                 start=True, stop=True)
                nc.tensor.matmul(pl1[:, 512:1024], tw1[:, 128:256], xs,
                                 start=True, stop=True)
                h1 = actp.tile([128, 1024], f32r, tag="h1")
                nc.scalar.activation(h1[:], pl1[:], AF.Relu)
                st[i] = {"h1": h1}

            def emit_l2(i):
                h1 = st[i].pop("h1")
                pl2 = ps2.tile([128, 1024], f32, tag="l2")
                for m in (0, 1):
                    ms = slice(128 * m, 128 * (m + 1))
                    os_ = slice(512 * m, 512 * (m + 1))
                    nc.tensor.matmul(pl2[:, os_], tw2a[:, ms],
                                     h1[:, 0:512], start=True, stop=False)
                    nc.tensor.matmul(pl2[:, os_], tw2b[:, ms],
                                     h1[:, 512:1024], start=False, stop=True)
                h2 = actp.tile([128, 1024], f32r, tag="h2")
                nc.vector.tensor_scalar(h2[:, 0:512], pl2[:, 0:512],
                                        tsb2a[:], 0.0, OP.add, OP.max)
                nc.vector.tensor_scalar(h2[:, 512:1024], pl2[:, 512:1024],
                                        tsb2b[:], 0.0, OP.add, OP.max)
                st[i]["h2"] = h2

            def emit_l3(i):
                h2 = st.pop(i)["h2"]
                s, blk = (512 * i) // _Bc, ((512 * i) % _Bc) // _CH
                pl3 = ps3.tile([128, 512], f32, tag="l3")
                nc.tensor.matmul(pl3[:], tw3a[:], h2[:, 0:512],
                                 start=True, stop=False)
                nc.tensor.matmul(pl3[:], tw3b[:], h2[:, 512:1024],
                                 start=False, stop=True)
                dst = pooled[:, blk * _CH:(blk + 1) * _CH]
                if s == 0:
                    nc.scalar.activation(dst, pl3[:], AF.Relu, bias=tsb3c[:])
                else:
                    h3 = actp.tile([128, 512], f32r, tag="h3")
                    if i % evm >= evk:
                        nc.vector.tensor_scalar(h3[:], pl3[:], tsb3c[:],
                                                0.0, OP.add, OP.max)
                    else:
                        nc.scalar.activation(h3[:], pl3[:], AF.Relu,
                                             bias=tsb3c[:])
                    nc.gpsimd.tensor_tensor(dst, dst, h3[:], op=OP.add)

            for i in range(NCH + 2):
                if i < NCH:
                    emit_l1(i)
                if i == GRP:
                    emit_phase_a()
                if i - 1 >= 0 and i - 1 < NCH:
                    emit_l2(i - 1)
                if i - 2 >= 0:
                    emit_l3(i - 2)

            # ---- correction, then software-pipelined heads ----
            for blk in range(_Bc // _CH):
                bs = slice(blk * _CH, (blk + 1) * _CH)
                psc = ps3.tile([128, 512], f32, tag="l3")
                nc.tensor.matmul(psc[:], ct[:], tnneg[:, bs],
                                 start=True, stop=True)
                nc.vector.tensor_tensor(pooled[:, bs], pooled[:, bs], psc[:],
                                        op=OP.add)

            acfg = (W("aw1x"), W("aw1o"), W("aw2a"), W("aw2b"),
                    Bv("ab2a"), Bv("ab2b"), W("aw3a"), W("aw3b"),
                    Bv("ab3c"), _NOUT)
            vcfg = (W("vw1x"), W("vw1o"), W("vw2a"), W("vw2b"),
                    Bv("vb2a"), Bv("vb2b"), W("vw3a"), W("vw3b"),
                    Bv("vb3c"), 1)
            units = [(blk, cfg) for blk in range(_Bc // _CH)
                     for cfg in (acfg, vcfg)]
            hst = {}

            def emit_h1(j):
                blk, cfg = units[j]
                bs = slice(blk * _CH, (blk + 1) * _CH)
                w1x, w1o = cfg[0], cfg[1]
                pA = ps1.tile([128, 1024], f32, tag="l1")
                for m in (0, 1):
                    ms = slice(128 * m, 128 * (m + 1))
                    os_ = slice(512 * m, 512 * (m + 1))
                    nc.tensor.matmul(pA[:, os_], w1x[:, ms],
                                     pooled[:, bs], start=True, stop=False)
                    nc.tensor.matmul(pA[:, os_], w1o[:, ms],
                                     town[:, bs], start=False, stop=True)
                s1 = actp.tile([128, 1024], f32r, tag="h1")
                nc.scalar.activation(s1[:], pA[:], AF.Relu)
                hst[j] = {"s1": s1}

            def emit_h2(j):
                blk, cfg = units[j]
                hw2a, hw2b, hb2a, hb2b = cfg[2], cfg[3], cfg[4], cfg[5]
                s1 = hst[j].pop("s1")
                pB = ps2.tile([128, 1024], f32, tag="l2")
                for m in (0, 1):
                    ms = slice(128 * m, 128 * (m + 1))
                    os_ = slice(512 * m, 512 * (m + 1))
                    nc.tensor.matmul(pB[:, os_], hw2a[:, ms],
                                     s1[:, 0:512], start=True, stop=False)
                    nc.tensor.matmul(pB[:, os_], hw2b[:, ms],
                                     s1[:, 512:1024], start=False, stop=True)
                s2 = actp.tile([128, 1024], f32r, tag="h2")
                nc.vector.tensor_scalar(s2[:, 0:512], pB[:, 0:512],
                                        hb2a[:], 0.0, OP.add, OP.max)
                nc.vector.tensor_scalar(s2[:, 512:1024], pB[:, 512:1024],
                                        hb2b[:], 0.0, OP.add, OP.max)
                hst[j]["s2"] = s2

            def emit_h3(j):
                blk, cfg = units[j]
                bs = slice(blk * _CH, (blk + 1) * _CH)
                hw3a, hw3b, hb3, onum = cfg[6], cfg[7], cfg[8], cfg[9]
                s2 = hst.pop(j)["s2"]
                pC = ps3.tile([onum, 512], f32, tag="l3")
                nc.tensor.matmul(pC[:], hw3a[:, 0:onum], s2[:, 0:512],
                                 start=True, stop=False)
                nc.tensor.matmul(pC[:], hw3b[:, 0:onum], s2[:, 512:1024],
                                 start=False, stop=True)
                odst = outT if onum == _NOUT else outV
                nc.scalar.activation(odst[0:onum, bs], pC[:],
                                     AF.Identity, bias=hb3[:])

            NU = len(units)
            for j in range(NU + 2):
                if j < NU:
                    emit_h1(j)
                if j - 1 >= 0 and j - 1 < NU:
                    emit_h2(j - 1)
                if j - 2 >= 0:
                    emit_h3(j - 2)

            nc.sync.dma_start(out[:, :], outT[:])
            nc.sync.dma_start(outv[:, :], outV[:])

    nc.finalize()
    return nc


def _host_prep(inputs):
    ins = {k: np.ascontiguousarray(np.asarray(v), dtype=np.float32)
           for k, v in inputs.items()}
    obs = ins["obs"]
    seqs = obs[:, 1:, :]
    own = obs[:, 0, :_OWN]
    valid = (np.abs(seqs).sum(-1) != 0)
    negnpad = -(_S - valid.sum(1)).astype(np.float32)

    wvals = {
        "w1": np.concatenate([ins["sW1"], ins["sb1"][None, :]], 0),
        "w2a": ins["sW2"][:128], "w2b": ins["sW2"][128:],
        "w3a": ins["sW3"][:128], "w3b": ins["sW3"][128:],
        "aw1x": ins["aW1"][:128],
        "aw1o": np.concatenate([ins["aW1"][128:], ins["ab1"][None, :]], 0),
        "vw1x": ins["vW1"][:128],
        "vw1o": np.concatenate([ins["vW1"][128:], ins["vb1"][None, :]], 0),
        "aw2a": ins["aW2"][:128], "aw2b": ins["aW2"][128:],
        "vw2a": ins["vW2"][:128], "vw2b": ins["vW2"][128:],
        "aw3a": ins["aW3"][:128], "aw3b": ins["aW3"][128:],
        "vw3a": ins["vW3"][:128], "vw3b": ins["vW3"][128:],
    }
    bvals = {
        "sb1a": ins["sb1"][:128, None], "sb1b": ins["sb1"][128:, None],
        "sb2a": ins["sb2"][:128, None], "sb2b": ins["sb2"][128:, None],
        "sb3c": ins["sb3"][:, None],
        "ab2a": ins["ab2"][:128, None], "ab2b": ins["ab2"][128:, None],
        "vb2a": ins["vb2"][:128, None], "vb2b": ins["vb2"][128:, None],
        "ab3c": ins["ab3"][:, None], "vb3c": ins["vb3"][:, None],
        "sb3r": ins["sb3"][None, :],
        "sb1a2": np.repeat(ins["sb1"][:128, None], 2, axis=1),
        "sb1b2": np.repeat(ins["sb1"][128:, None], 2, axis=1),
    }

    bblob = np.zeros((128, _BTOT), np.float32)
    for name, (c, p, w) in _BOFF.items():
        bblob[:p, c:c + w] = bvals[name]

    wblob = np.zeros((128, _WTOT), np.float32)
    for name, (c, p, w) in _WOFF.items():
        wblob[:p, c:c + w] = wvals[name]
    w1d = np.ascontiguousarray(wvals["w1"])

    in_maps = []
    for cid in range(_NCORES):
        sl = slice(cid * _Bc, (cid + 1) * _Bc)
        xT = np.empty((18, _R), np.float32)
        xT[:17] = seqs[sl].transpose(2, 1, 0).reshape(17, _R)
        xT[17] = 1.0

        hblob = np.zeros((10, 2 * _Bc), np.float32)
        hblob[:9, :_Bc] = own[sl].T
        hblob[9, :_Bc] = 1.0
        hblob[0, _Bc:] = negnpad[sl]
        in_maps.append({"x": xT, "w1d": w1d, "wblob": wblob,
                        "bblob": bblob, "hblob": hblob})
    return in_maps


def _get_exec(nreps=1):
    """Build (once per nreps) a jitted shard_map executing the bass NEFF on
    8 cores; nreps>1 chains the custom call for device-time benchmarking."""
    global _EXEC, _NC_CACHE
    if _EXEC is None:
        _EXEC = {}
    if nreps in _EXEC:
        return _EXEC[nreps]
    import jax
    from jax.sharding import Mesh, NamedSharding, PartitionSpec
    from jax.experimental.shard_map import shard_map
    from concourse import bass2jax, mybir

    bass2jax.install_neuronx_cc_hook()
    if _NC_CACHE is None:
        _NC_CACHE = _build()
    nc = _NC_CACHE

    in_names, out_names, out_avals, zero_outs = [], [], [], []
    for alloc in nc.m.functions[0].allocations:
        if not isinstance(alloc, mybir.MemoryLocationSet):
            continue
        name = alloc.memorylocations[0].name
        if alloc.kind == "ExternalInput":
            in_names.append(name)
        elif alloc.kind == "ExternalOutput":
            out_names.append(name)
            shape = tuple(alloc.tensor_shape)
            dtype = mybir.dt.np(alloc.dtype)
            out_avals.append(jax.core.ShapedArray(shape, dtype))
            zero_outs.append(np.zeros(shape, dtype))
    all_names = tuple(in_names + out_names)

    def _body(*args):
        for _ in range(nreps):
            outs = bass2jax._bass_exec_p.bind(
                *args,
                out_avals=tuple(out_avals),
                in_names=all_names,
                out_names=tuple(out_names),
                lowering_input_output_aliases=(),
                sim_require_finite=True,
                sim_require_nnan=True,
                nc=nc,
            )
        return tuple(outs)

    devices = jax.devices()[:_NCORES]
    mesh = Mesh(np.asarray(devices), ("core",))
    nin = len(in_names) + len(out_names)
    fn = jax.jit(
        shard_map(_body, mesh=mesh,
                  in_specs=(PartitionSpec("core"),) * nin,
                  out_specs=(PartitionSpec("core"),) * len(out_names),
                  check_rep=False),
        keep_unused=True)
    shard = NamedSharding(mesh, PartitionSpec("core"))
    _EXEC[nreps] = (fn, in_names, out_names, zero_outs, shard)
    return _EXEC[nreps]


def _device_inputs(in_maps):
    import jax
    fn, in_names, out_names, zero_outs, shard = _get_exec()
    concat = [np.concatenate([np.asarray(m[n]) for m in in_maps], axis=0)
              for n in in_names]
    concat += [np.concatenate([z] * _NCORES, axis=0) for z in zero_outs]
    return [jax.device_put(a, shard) for a in concat]


def _run_device(args_dev):
    fn, in_names, out_names, zero_outs, shard = _get_exec()
    outs = fn(*args_dev)
    return {n: np.asarray(o) for n, o in zip(out_names, outs)}


def kernel(**inputs):
    in_maps = _host_prep(inputs)
    args_dev = _device_inputs(in_maps)
    outs = _run_device(args_dev)
    o = outs["out"].reshape(_NCORES, _NOUT, _Bc)
    ov = outs["outv"].reshape(_NCORES, 1, _Bc)
    logits = np.concatenate([o[c].T for c in range(_NCORES)], 0)
    value = np.concatenate([ov[c, 0] for c in range(_NCORES)], 0)
    return (np.ascontiguousarray(logits, dtype=np.float32),
            np.ascontiguousarray(value, dtype=np.float32))


def _time_fn(fn, args_dev, reps):
    import time
    ts = []
    for _ in range(reps + 1):
        t0 = time.perf_counter()
        outs = fn(*args_dev)
        for o in outs:
            o.block_until_ready()
        ts.append(time.perf_counter() - t0)
    return ts[1:]


def benchmark(inputs, reps=20):
    """Warm per-call wall-clock times (seconds) of the device execution."""
    in_maps = _host_prep(inputs)
    args_dev = _device_inputs(in_maps)
    fn = _get_exec()[0]
    return _time_fn(fn, args_dev, reps)


def benchmark_device(inputs, nreps=16, outer=6):
    """Estimate per-NEFF-execution device time by differencing a 1-exec jit
    against an nreps-exec jit (dispatch overhead cancels)."""
    in_maps = _host_prep(inputs)
    args_dev = _device_inputs(in_maps)
    fn1 = _get_exec(1)[0]
    fnN = _get_exec(nreps)[0]
    t1 = min(_time_fn(fn1, args_dev, outer))
    tn = min(_time_fn(fnN, args_dev, outer))
    per_exec = (tn - t1) / (nreps - 1)
    return per_exec, t1, tn
